# revision 7
# baseline (speedup 1.0000x reference)
"""AttentionBlock (GroupNorm + single-head full attention + residual) on 8 TRN2 cores.

Data-parallel: batch B=8, one sample per NeuronCore. Per core:
  x [256, 4096] f32 -> groupnorm -> h (fp8 e4m3)
  Algebraic folding (host-precomputed weight products):
    S[q,k] = q.k = sum_c h[c,q]*G2[c,k] + w[k] + c0
       G2 = M h + v,  M = Wq^T Wk, v = Wq^T b_k,  w[k] = (Wk^T b_q).h_k
    out_pre[q,co] = sum_k P[k,q]*VV[co,k],  VV = (Wo Wv) h   (proj_out folded;
       the Wo b_v term rides on the output bias since sum_k softmax = 1, and
       the c0 score offset cancels in softmax)
  All heavy matmuls run in fp8 e4m3 with MatmulPerfMode.DoubleRow (contraction
  over 2 k-subtiles per instruction, 2x PE throughput).  The per-k score bias
  w[k] is folded multiplicatively into VV (f[k] = exp(SCALE*w[k]),
  sum_k e*f*vv == sum_k (e*f)*vv), which makes the softmax-exp bias a constant
  (-SHIFT) so each ACT exp instruction can span two PSUM banks (1024 wide).
  The softmax denominator rides as an f-column of VV.  P^T layout [k, q] comes
  straight out of the S^T matmul so the 4096x4096 attention matrix is never
  transposed; only the final [4096, 256] attention output is transposed back
  to [c, n] via TensorE.
"""

import numpy as np
import ml_dtypes

import concourse.bacc as bacc
import concourse.bass as bass
import concourse.tile as tile
from concourse import mybir
from concourse.bass_utils import run_bass_kernel_spmd

F32 = mybir.dt.float32
BF16 = mybir.dt.bfloat16
F8 = mybir.dt.float8e4
AF = mybir.ActivationFunctionType
DR = mybir.MatmulPerfMode.DoubleRow
ALU = mybir.AluOpType
F8NP = ml_dtypes.float8_e4m3fn

C = 256          # channels
N = 4096         # spatial (64*64)
P = 128          # partitions
CT = C // P      # channel tiles (2)
NG = 8           # groups
GS = C // NG     # group size (32)
EPS = 1e-5
QB = 512         # queries per block
NQB = N // QB    # 8
NKT = N // P     # 32 k-tiles
NPR = NKT // 2   # 16 k-tile pairs
SCALE = 1.0 / np.sqrt(C)  # 1/16
SHIFT = 3.0      # global exp shift (softmax-invariant), keeps fp8 e in range


def _group_masks():
    g0 = np.zeros((P, NG), np.float32)
    g1 = np.zeros((P, NG), np.float32)
    for p in range(P):
        g0[p, p // GS] = 1.0
        g1[p, 4 + p // GS] = 1.0
    return g0, g1


def build_nc():
    nc = bacc.Bacc("TRN2", target_bir_lowering=False)

    x_d = nc.dram_tensor("x", [C, N], F32, kind="ExternalInput")
    mt8_d = nc.dram_tensor("mt8", [P, CT, C], F8, kind="ExternalInput")
    vb_d = nc.dram_tensor("vb", [P, CT], F32, kind="ExternalInput")
    w2t8_d = nc.dram_tensor("w2t8", [P, CT, 257], F8, kind="ExternalInput")
    bo_d = nc.dram_tensor("bo", [P, CT], F32, kind="ExternalInput")
    out_d = nc.dram_tensor("out", [C, N], F32, kind="ExternalOutput")

    g0_np, g1_np = _group_masks()
    g0_d = nc.inline_tensor(g0_np, name="g0c")
    g1_d = nc.inline_tensor(g1_np, name="g1c")
    gt0_d = nc.inline_tensor(np.ascontiguousarray(g0_np.T), name="gt0c")
    gt1_d = nc.inline_tensor(np.ascontiguousarray(g1_np.T), name="gt1c")
    eye_d = nc.inline_tensor(np.eye(P, dtype=np.float32), name="eyec")

    import contextlib
    with tile.TileContext(nc) as tc, contextlib.ExitStack() as ctx:
        cst = ctx.enter_context(tc.tile_pool(name="cst", bufs=1))
        big = ctx.enter_context(tc.tile_pool(name="big", bufs=1))
        e4p = ctx.enter_context(tc.tile_pool(name="e4p", bufs=2))
        anp = ctx.enter_context(tc.tile_pool(name="anp", bufs=4))
        outp = ctx.enter_context(tc.tile_pool(name="outp", bufs=2))
        sml = ctx.enter_context(tc.tile_pool(name="sml", bufs=2))
        tpp = ctx.enter_context(tc.tile_pool(name="tpp", bufs=4))
        ps_s = ctx.enter_context(tc.tile_pool(name="ps_s", bufs=3, space="PSUM"))
        ps_o = ctx.enter_context(tc.tile_pool(name="ps_o", bufs=2, space="PSUM"))

        # ---- const loads ----
        mt8_sb = cst.tile([P, CT, C], F8, name="mt8_sb")
        nc.sync.dma_start(out=mt8_sb, in_=mt8_d[:, :, :])
        w2t8_sb = cst.tile([P, CT, 257], F8, name="w2t8_sb")
        nc.sync.dma_start(out=w2t8_sb, in_=w2t8_d[:, :, :])
        vb_sb = cst.tile([P, CT], F32, name="vb_sb")
        nc.sync.dma_start(out=vb_sb, in_=vb_d[:, :])
        bo_sb = cst.tile([P, CT], F32, name="bo_sb")
        nc.sync.dma_start(out=bo_sb, in_=bo_d[:, :])

        eye_sb = cst.tile([P, P], F32, name="eye_sb")
        nc.sync.dma_start(out=eye_sb, in_=eye_d[:, :])
        eyeb = cst.tile([P, P], BF16, name="eyeb")
        nc.vector.tensor_copy(out=eyeb, in_=eye_sb)

        g0_sb = cst.tile([P, NG], F32, name="g0_sb")
        nc.sync.dma_start(out=g0_sb, in_=g0_d[:, :])
        g1_sb = cst.tile([P, NG], F32, name="g1_sb")
        nc.sync.dma_start(out=g1_sb, in_=g1_d[:, :])
        gt0_sb = cst.tile([NG, P], F32, name="gt0_sb")
        nc.sync.dma_start(out=gt0_sb, in_=gt0_d[:, :])
        gt1_sb = cst.tile([NG, P], F32, name="gt1_sb")
        nc.sync.dma_start(out=gt1_sb, in_=gt1_d[:, :])

        eps_sb = cst.tile([NG, 1], F32, name="eps_sb")
        nc.vector.memset(eps_sb, EPS)
        warm = cst.tile([NG, 1], F32, name="warm")
        nc.scalar.activation(out=warm, in_=eps_sb, func=AF.Sqrt, bias=eps_sb)
        nshift = cst.tile([P, 1], F32, name="nshift")
        nc.vector.memset(nshift, -SHIFT)
        zbias = cst.tile([P, 1], F32, name="zbias")
        nc.vector.memset(zbias, 0.0)

        # ---- x load (4 chunks to parallelize DMA queues) ----
        x_sb = big.tile([P, CT, N], F32, name="x_sb")
        x_r = x_d.rearrange("(t p) n -> p t n", p=P)
        NXC = 8  # chunks per ct
        XC = N // NXC
        for xc in range(NXC):
            for ct in range(CT):
                xs = slice(xc * XC, (xc + 1) * XC)
                nc.sync.dma_start(out=x_sb[:, ct, xs], in_=x_r[:, ct, xs])

        # ---- groupnorm stats ----
        NSG = N // 512
        stats = sml.tile([P, CT, NSG, 6], F32, name="stats")
        mv = sml.tile([P, CT, 2], F32, name="mv")
        for ct in range(CT):
            for sg in range(NSG):
                nc.vector.bn_stats(
                    out=stats[:, ct, sg, :], in_=x_sb[:, ct, sg * 512:(sg + 1) * 512]
                )
            nc.vector.bn_aggr(out=mv[:, ct, :], in_=stats[:, ct, :, :])
        st3 = sml.tile([P, CT, 3], F32, name="st3")
        for ct in range(CT):
            nc.vector.tensor_copy(out=st3[:, ct, 0:2], in_=mv[:, ct, :])
            nc.vector.tensor_mul(
                out=st3[:, ct, 2:3], in0=mv[:, ct, 0:1], in1=mv[:, ct, 0:1]
            )
        gps = ps_s.tile([NG, 3], F32, name="gps", tag="s")
        nc.tensor.matmul(gps, lhsT=g0_sb, rhs=st3[:, 0, :], start=True, stop=False)
        nc.tensor.matmul(gps, lhsT=g1_sb, rhs=st3[:, 1, :], start=False, stop=True)
        gsb = sml.tile([NG, 3], F32, name="gsb")
        nc.vector.tensor_copy(out=gsb, in_=gps)
        gmean = sml.tile([NG, 1], F32, name="gmean")
        nc.vector.tensor_scalar_mul(out=gmean, in0=gsb[:, 0:1], scalar1=1.0 / GS)
        gtmp = sml.tile([NG, 1], F32, name="gtmp")
        nc.vector.tensor_add(out=gtmp, in0=gsb[:, 1:2], in1=gsb[:, 2:3])
        nc.vector.tensor_scalar_mul(out=gtmp, in0=gtmp, scalar1=1.0 / GS)
        gmsq = sml.tile([NG, 1], F32, name="gmsq")
        nc.vector.tensor_mul(out=gmsq, in0=gmean, in1=gmean)
        gvar = sml.tile([NG, 1], F32, name="gvar")
        nc.vector.tensor_sub(out=gvar, in0=gtmp, in1=gmsq)
        gstd = sml.tile([NG, 1], F32, name="gstd")
        nc.scalar.activation(out=gstd, in_=gvar, func=AF.Sqrt, bias=eps_sb)
        grstd = sml.tile([NG, 1], F32, name="grstd")
        nc.vector.reciprocal(out=grstd, in_=gstd)
        gpar = sml.tile([NG, 2], F32, name="gpar")
        nc.vector.tensor_copy(out=gpar[:, 0:1], in_=gmean)
        nc.vector.tensor_copy(out=gpar[:, 1:2], in_=grstd)
        mr_sb = sml.tile([P, CT, 2], F32, name="mr_sb")
        for ct, gt in ((0, gt0_sb), (1, gt1_sb)):
            bps = ps_s.tile([P, 2], F32, name=f"bps{ct}", tag="s")
            nc.tensor.matmul(bps, lhsT=gt, rhs=gpar, start=True, stop=True)
            nc.vector.tensor_copy(out=mr_sb[:, ct, :], in_=bps)

        # ---- h fp8 + G2, pipelined per 512-col block ----
        hf8 = big.tile([P, CT, N], F8, name="hf8")
        g2f8 = big.tile([P, CT, N], F8, name="g2f8")
        for kb in range(NQB):
            ks = slice(kb * QB, (kb + 1) * QB)
            for ct in range(CT):
                nc.gpsimd.tensor_scalar(
                    out=hf8[:, ct, ks],
                    in0=x_sb[:, ct, ks],
                    scalar1=mr_sb[:, ct, 0:1],
                    scalar2=mr_sb[:, ct, 1:2],
                    op0=ALU.subtract,
                    op1=ALU.mult,
                )
            for ct in range(CT):
                g2ps = ps_s.tile([P, QB], F32, name=f"g2ps_{kb}_{ct}", tag="s")
                nc.tensor.matmul(
                    g2ps, lhsT=mt8_sb[:, :, ct * P:(ct + 1) * P],
                    rhs=hf8[:, :, ks], start=True, stop=True, perf_mode=DR,
                )
                nc.scalar.activation(
                    out=g2f8[:, ct, ks], in_=g2ps, func=AF.Identity,
                    scale=1.0 / 16.0, bias=vb_sb[:, ct:ct + 1],
                )

        # ---- VV projection + f = exp(SCALE*w) folding ----
        # vv8[k, 0:256] = 16*VV[k, co]*f[k]; vv8[k, 256] = f[k]
        vv8 = big.tile([P, NKT, 257], F8, name="vv8")
        fz = big.tile([P, NKT], F32, name="fz")
        ftmp = big.tile([P, NKT], F32, name="ftmp")
        fex = big.tile([P, NKT], F32, name="fex")

        def emit_vv_pair(i):
            kt0, kt1 = 2 * i, 2 * i + 1
            vps = {}
            for kt in (kt0, kt1):
                vps[kt] = ps_o.tile([P, 257], F32, name=f"vps_{kt}", tag="o")
                ks = slice(kt * P, (kt + 1) * P)
                nc.tensor.matmul(vps[kt], lhsT=hf8[:, :, ks], rhs=w2t8_sb,
                                 start=True, stop=True, perf_mode=DR)
            for kt in (kt0, kt1):
                nc.vector.tensor_scalar_mul(
                    out=fz[:, kt:kt + 1], in0=vps[kt][:, 256:257],
                    scalar1=float(SCALE / 16.0),
                )
            pr = slice(kt0, kt1 + 1)
            nc.vector.tensor_scalar(
                out=ftmp[:, pr], in0=fz[:, pr], scalar1=0.5, scalar2=1.0,
                op0=ALU.mult, op1=ALU.add,
            )
            nc.vector.tensor_mul(out=fex[:, pr], in0=ftmp[:, pr], in1=fz[:, pr])
            nc.vector.tensor_scalar_add(out=fex[:, pr], in0=fex[:, pr], scalar1=1.0)
            for kt in (kt0, kt1):
                nc.vector.tensor_scalar_mul(
                    out=vv8[:, kt, 0:256], in0=vps[kt][:, 0:256],
                    scalar1=fex[:, kt:kt + 1],
                )
                nc.vector.tensor_scalar_mul(
                    out=vv8[:, kt, 256:257], in0=fex[:, kt:kt + 1], scalar1=1.0
                )

        # ---- attention ----
        e4_tiles = {}
        o_cur = {}
        tps_cur = {}

        def emit_s_pair(qb, t):
            qs_ = slice(qb * QB, (qb + 1) * QB)
            sp = ps_s.tile([P, 2, QB], F32, name=f"sps_{qb}_{t}", tag="s")
            for j in (0, 1):
                kt = 2 * t + j
                nc.tensor.matmul(
                    sp[:, j, :], lhsT=g2f8[:, :, kt * P:(kt + 1) * P],
                    rhs=hf8[:, :, qs_], start=True, stop=True, perf_mode=DR,
                )
            nc.scalar.activation(
                out=e4_tiles[qb][:, 2 * t:2 * t + 2, :], in_=sp,
                func=AF.Exp, scale=float(SCALE), bias=nshift,
            )

        def emit_pv(qb, qs, t):
            if t == 0:
                o_cur[qs] = ps_o.tile([P, 257], F32, name=f"ops_{qb}_{qs}", tag="o")
            nc.tensor.matmul(
                o_cur[qs],
                lhsT=e4_tiles[qb][:, 2 * t:2 * t + 2, qs * P:(qs + 1) * P],
                rhs=vv8[:, 2 * t:2 * t + 2, :],
                start=(t == 0), stop=(t == NPR - 1), perf_mode=DR,
            )

        def emit_qs_epilogue(qb, qs):
            o = o_cur[qs]
            recip = sml.tile([P, 1], F32, name=f"rc_{qb}_{qs}", tag="recip")
            nc.vector.reciprocal(out=recip, in_=o[:, 256:257])
            recip2 = sml.tile([P, 1], F32, name=f"rc2_{qb}_{qs}", tag="recip2")
            nc.vector.tensor_scalar_mul(out=recip2, in0=recip, scalar1=1.0 / 16.0)
            attn = anp.tile([P, C], BF16, name=f"attn_{qb}_{qs}", tag="attn")
            nc.vector.tensor_scalar_mul(out=attn, in0=o[:, 0:256], scalar1=recip2)
            tps = tpp.tile([P, CT, P], BF16, name=f"tps_{qb}_{qs}", tag="t")
            tps_cur[qs] = tps
            for ct in range(CT):
                nc.sync.dma_start_transpose(
                    out=tps[:, ct, :], in_=attn[:, ct * P:(ct + 1) * P],
                )
            # out tile slice for this qs
            outt = outt_cur[qb]
            n0 = qb * QB + qs * P
            for ct in range(CT):
                nc.vector.tensor_scalar_add(
                    out=outt[:, ct, qs * P:(qs + 1) * P], in0=tps[:, ct, :],
                    scalar1=bo_sb[:, ct:ct + 1],
                )
                nc.vector.tensor_add(
                    out=outt[:, ct, qs * P:(qs + 1) * P],
                    in0=outt[:, ct, qs * P:(qs + 1) * P],
                    in1=x_sb[:, ct, n0:n0 + P],
                )

        outt_cur = {}

        def emit_qb_out(qb):
            outt = outt_cur.pop(qb)
            qs_ = slice(qb * QB, (qb + 1) * QB)
            out_r = out_d.rearrange("(t p) n -> p t n", p=P)
            nc.gpsimd.dma_start(out=out_r[:, :, qs_], in_=outt)

        def aux_pv(qb_prev, i):  # i in 0..15 -> 4 PV matmuls per step
            for k in range(4):
                idx = 4 * i + k
                qs, t = divmod(idx, NPR)
                if qs == 0 and t == 0:
                    outt_cur[qb_prev] = outp.tile(
                        [P, CT, QB], F32, name=f"outt_{qb_prev}", tag="outt"
                    )
                emit_pv(qb_prev, qs, t)
                if t == NPR - 1:
                    emit_qs_epilogue(qb_prev, qs)
                    if qs == 3:
                        emit_qb_out(qb_prev)

        for qb in range(NQB):
            e4_tiles[qb] = e4p.tile([P, NKT, QB], F8, name=f"e4_{qb}", tag="e4")
            if qb >= 2:
                del e4_tiles[qb - 2]
            for t in range(NPR):
                emit_s_pair(qb, t)
                if qb == 0:
                    emit_vv_pair(t)
                else:
                    aux_pv(qb - 1, t)
        for i in range(NPR):
            aux_pv(NQB - 1, i)

    nc.compile()
    return nc


_NC = None


def _get_nc():
    global _NC
    if _NC is None:
        _NC = build_nc()
    return _NC


def _host_prep(x, w_q, b_q, w_k, b_k, w_v, b_v, w_o, b_o):
    x = np.ascontiguousarray(np.asarray(x, np.float32))
    B = x.shape[0]
    wq = np.asarray(w_q, np.float32)
    wk = np.asarray(w_k, np.float32)
    wv = np.asarray(w_v, np.float32)
    wo = np.asarray(w_o, np.float32)
    bq = np.asarray(b_q, np.float32)
    bk = np.asarray(b_k, np.float32)
    bv = np.asarray(b_v, np.float32)
    bo = np.asarray(b_o, np.float32)

    def to_pt(a):  # [C, ...] -> [P, CT, ...]
        return np.ascontiguousarray(
            a.reshape(CT, P, *a.shape[1:]).transpose(1, 0, *range(2, a.ndim + 1))
        )

    mt = (wk.T @ wq).astype(np.float32)       # lhsT[c, c'] = M[c', c]
    mt8 = to_pt((16.0 * mt).astype(F8NP))
    v = (wq.T @ bk).astype(np.float32)
    u = (wk.T @ bq).astype(np.float32)
    w2 = (wo @ wv).astype(np.float32)
    b2 = (wo @ bv).astype(np.float32)
    w2t = np.zeros((C, 257), np.float32)
    w2t[:, :256] = 16.0 * w2.T
    w2t[:, 256] = 16.0 * u
    w2t8 = to_pt(w2t.astype(F8NP))
    bo_eff = bo + b2   # sum_k softmax = 1 -> Wo b_v folds into the output bias

    xr = x.reshape(B, C, N)
    shared = {
        "mt8": mt8, "vb": to_pt(v), "w2t8": w2t8, "bo": to_pt(bo_eff),
    }
    in_maps = [{"x": np.ascontiguousarray(xr[i]), **shared} for i in range(B)]
    return x, in_maps


def kernel(x, w_q, b_q, w_k, b_k, w_v, b_v, w_o, b_o):
    x, in_maps = _host_prep(x, w_q, b_q, w_k, b_k, w_v, b_v, w_o, b_o)
    B = x.shape[0]
    nc = _get_nc()
    res = run_bass_kernel_spmd(nc, in_maps, core_ids=list(range(B)))
    global _LAST
    _LAST = res
    out = np.stack([res.results[i]["out"] for i in range(B)], axis=0)
    return out.reshape(x.shape).astype(np.float32)


_LAST = None


# revision 8
# speedup vs baseline: 1.2239x; 1.2239x over previous
"""AttentionBlock (GroupNorm + single-head full attention + residual) on 8 TRN2 cores.

Data-parallel: batch B=8, one sample per NeuronCore. Per core:
  x [256, 4096] f32 -> groupnorm -> h (fp8 e4m3)
  Algebraic folding (host-precomputed weight products):
    S[q,k] = q.k = sum_c h[c,q]*G2[c,k] + w[k] (+ per-q terms that cancel in
       softmax), G2 = M h, M = Wq^T Wk, w[k] = (Wk^T b_q).h_k
    out_pre[q,co] = sum_k P[k,q]*VV[co,k],  VV = (Wo Wv) h   (proj_out folded;
       the Wo b_v term rides on the output bias since sum_k softmax = 1)
  All heavy matmuls run in fp8 e4m3 with MatmulPerfMode.DoubleRow (contraction
  over 2 k-subtiles per instruction, 2x PE throughput).  The per-k score bias
  w[k] is folded multiplicatively into VV (f[k] = exp(SCALE*w[k]),
  sum_k e*f*vv == sum_k (e*f)*vv), which makes the softmax-exp bias a constant
  (-SHIFT) so each ACT exp instruction can span two PSUM banks (1024 wide).
  The softmax denominator rides as an f-column of VV.  P^T layout [k, q] comes
  straight out of the S^T matmul so the 4096x4096 attention matrix is never
  transposed; only the final [4096, 256] attention output is transposed back
  to [c, n] via TensorE.  The h-fp8 / G2 / VV production is pipelined into
  q-block 0's S phase so every engine starts hot.
"""

import numpy as np
import ml_dtypes

import concourse.bacc as bacc
import concourse.bass as bass
import concourse.tile as tile
from concourse import mybir
from concourse.bass_utils import run_bass_kernel_spmd

F32 = mybir.dt.float32
BF16 = mybir.dt.bfloat16
F8 = mybir.dt.float8e4
AF = mybir.ActivationFunctionType
DR = mybir.MatmulPerfMode.DoubleRow
ALU = mybir.AluOpType
F8NP = ml_dtypes.float8_e4m3fn

C = 256          # channels
N = 4096         # spatial (64*64)
P = 128          # partitions
CT = C // P      # channel tiles (2)
NG = 8           # groups
GS = C // NG     # group size (32)
EPS = 1e-5
QB = 512         # queries per block
NQB = N // QB    # 8
NKT = N // P     # 32 k-tiles
NPR = NKT // 2   # 16 k-tile pairs
SCALE = 1.0 / np.sqrt(C)  # 1/16
SHIFT = 3.0      # global exp shift (softmax-invariant), keeps fp8 e in range


def _group_masks():
    g0 = np.zeros((P, NG), np.float32)
    g1 = np.zeros((P, NG), np.float32)
    for p in range(P):
        g0[p, p // GS] = 1.0
        g1[p, 4 + p // GS] = 1.0
    return g0, g1


def build_nc():
    nc = bacc.Bacc("TRN2", target_bir_lowering=False)

    x_d = nc.dram_tensor("x", [C, N], F32, kind="ExternalInput")
    mt8_d = nc.dram_tensor("mt8", [P, CT, C], F8, kind="ExternalInput")
    w2t8_d = nc.dram_tensor("w2t8", [P, CT, 257], F8, kind="ExternalInput")
    bo_d = nc.dram_tensor("bo", [P, CT], F32, kind="ExternalInput")
    out_d = nc.dram_tensor("out", [C, N], F32, kind="ExternalOutput")

    g0_np, g1_np = _group_masks()
    g0_d = nc.inline_tensor(g0_np, name="g0c")
    g1_d = nc.inline_tensor(g1_np, name="g1c")
    gt0_d = nc.inline_tensor(np.ascontiguousarray(g0_np.T), name="gt0c")
    gt1_d = nc.inline_tensor(np.ascontiguousarray(g1_np.T), name="gt1c")
    eye_d = nc.inline_tensor(np.eye(P, dtype=np.float32), name="eyec")

    import contextlib
    with tile.TileContext(nc) as tc, contextlib.ExitStack() as ctx:
        cst = ctx.enter_context(tc.tile_pool(name="cst", bufs=1))
        big = ctx.enter_context(tc.tile_pool(name="big", bufs=1))
        e4p = ctx.enter_context(tc.tile_pool(name="e4p", bufs=2))
        anp = ctx.enter_context(tc.tile_pool(name="anp", bufs=4))
        outp = ctx.enter_context(tc.tile_pool(name="outp", bufs=2))
        sml = ctx.enter_context(tc.tile_pool(name="sml", bufs=2))
        ps_s = ctx.enter_context(tc.tile_pool(name="ps_s", bufs=2, space="PSUM"))
        ps_o = ctx.enter_context(tc.tile_pool(name="ps_o", bufs=4, space="PSUM"))

        # ---- const loads ----
        mt8_sb = cst.tile([P, CT, C], F8, name="mt8_sb")
        nc.sync.dma_start(out=mt8_sb, in_=mt8_d[:, :, :])
        w2t8_sb = cst.tile([P, CT, 257], F8, name="w2t8_sb")
        nc.sync.dma_start(out=w2t8_sb, in_=w2t8_d[:, :, :])
        bo_sb = cst.tile([P, CT], F32, name="bo_sb")
        nc.sync.dma_start(out=bo_sb, in_=bo_d[:, :])

        eye_sb = cst.tile([P, P], F32, name="eye_sb")
        nc.sync.dma_start(out=eye_sb, in_=eye_d[:, :])
        eyeb = cst.tile([P, P], BF16, name="eyeb")
        nc.vector.tensor_copy(out=eyeb, in_=eye_sb)

        g0_sb = cst.tile([P, NG], F32, name="g0_sb")
        nc.sync.dma_start(out=g0_sb, in_=g0_d[:, :])
        g1_sb = cst.tile([P, NG], F32, name="g1_sb")
        nc.sync.dma_start(out=g1_sb, in_=g1_d[:, :])
        gt0_sb = cst.tile([NG, P], F32, name="gt0_sb")
        nc.sync.dma_start(out=gt0_sb, in_=gt0_d[:, :])
        gt1_sb = cst.tile([NG, P], F32, name="gt1_sb")
        nc.sync.dma_start(out=gt1_sb, in_=gt1_d[:, :])

        eps_sb = cst.tile([NG, 1], F32, name="eps_sb")
        nc.vector.memset(eps_sb, EPS)
        warm = cst.tile([NG, 1], F32, name="warm")
        nc.scalar.activation(out=warm, in_=eps_sb, func=AF.Sqrt, bias=eps_sb)
        nshift = cst.tile([P, 1], F32, name="nshift")
        nc.vector.memset(nshift, -SHIFT)

        # ---- x load (16 chunks to parallelize DMA queues) ----
        x_sb = big.tile([P, CT, N], F32, name="x_sb")
        x_r = x_d.rearrange("(t p) n -> p t n", p=P)
        NXC = 8  # chunks per ct
        XC = N // NXC
        for xc in range(NXC):
            for ct in range(CT):
                xs = slice(xc * XC, (xc + 1) * XC)
                nc.sync.dma_start(out=x_sb[:, ct, xs], in_=x_r[:, ct, xs])

        # ---- groupnorm stats ----
        NSG = N // 512
        stats = sml.tile([P, CT, NSG, 6], F32, name="stats")
        mv = sml.tile([P, CT, 2], F32, name="mv")
        for sg in range(NSG):
            for ct in range(CT):
                nc.vector.bn_stats(
                    out=stats[:, ct, sg, :], in_=x_sb[:, ct, sg * 512:(sg + 1) * 512]
                )
        for ct in range(CT):
            nc.vector.bn_aggr(out=mv[:, ct, :], in_=stats[:, ct, :, :])
        st3 = sml.tile([P, CT, 3], F32, name="st3")
        for ct in range(CT):
            nc.vector.tensor_copy(out=st3[:, ct, 0:2], in_=mv[:, ct, :])
            nc.vector.tensor_mul(
                out=st3[:, ct, 2:3], in0=mv[:, ct, 0:1], in1=mv[:, ct, 0:1]
            )
        gps = ps_s.tile([NG, 3], F32, name="gps", tag="s")
        nc.tensor.matmul(gps, lhsT=g0_sb, rhs=st3[:, 0, :], start=True, stop=False)
        nc.tensor.matmul(gps, lhsT=g1_sb, rhs=st3[:, 1, :], start=False, stop=True)
        gsb = sml.tile([NG, 3], F32, name="gsb")
        nc.vector.tensor_copy(out=gsb, in_=gps)
        gmean = sml.tile([NG, 1], F32, name="gmean")
        nc.vector.tensor_scalar_mul(out=gmean, in0=gsb[:, 0:1], scalar1=1.0 / GS)
        gtmp = sml.tile([NG, 1], F32, name="gtmp")
        nc.vector.tensor_add(out=gtmp, in0=gsb[:, 1:2], in1=gsb[:, 2:3])
        nc.vector.tensor_scalar_mul(out=gtmp, in0=gtmp, scalar1=1.0 / GS)
        gmsq = sml.tile([NG, 1], F32, name="gmsq")
        nc.vector.tensor_mul(out=gmsq, in0=gmean, in1=gmean)
        gvar = sml.tile([NG, 1], F32, name="gvar")
        nc.vector.tensor_sub(out=gvar, in0=gtmp, in1=gmsq)
        gstd = sml.tile([NG, 1], F32, name="gstd")
        nc.scalar.activation(out=gstd, in_=gvar, func=AF.Sqrt, bias=eps_sb)
        grstd = sml.tile([NG, 1], F32, name="grstd")
        nc.vector.reciprocal(out=grstd, in_=gstd)
        gpar = sml.tile([NG, 2], F32, name="gpar")
        nc.vector.tensor_copy(out=gpar[:, 0:1], in_=gmean)
        nc.vector.tensor_copy(out=gpar[:, 1:2], in_=grstd)
        mr_sb = sml.tile([P, CT, 2], F32, name="mr_sb")
        for ct, gt in ((0, gt0_sb), (1, gt1_sb)):
            bps = ps_s.tile([P, 2], F32, name=f"bps{ct}", tag="s")
            nc.tensor.matmul(bps, lhsT=gt, rhs=gpar, start=True, stop=True)
            nc.vector.tensor_copy(out=mr_sb[:, ct, :], in_=bps)

        # ---- big tiles produced inside the qb0 pipeline ----
        hf8 = big.tile([P, CT, N], F8, name="hf8")
        g2f8 = big.tile([P, CT, N], F8, name="g2f8")
        vv8 = big.tile([P, NKT, 257], F8, name="vv8")
        fz = big.tile([P, NKT], F32, name="fz")
        ftmp = big.tile([P, NKT], F32, name="ftmp")
        fex = big.tile([P, NKT], F32, name="fex")

        def emit_hg2(kb):
            # h fp8 + G2 for one 512-col block
            ks = slice(kb * QB, (kb + 1) * QB)
            for ct in range(CT):
                nc.vector.tensor_scalar(
                    out=hf8[:, ct, ks],
                    in0=x_sb[:, ct, ks],
                    scalar1=mr_sb[:, ct, 0:1],
                    scalar2=mr_sb[:, ct, 1:2],
                    op0=ALU.subtract,
                    op1=ALU.mult,
                )
            for ct in range(CT):
                g2ps = ps_s.tile([P, QB], F32, name=f"g2ps_{kb}_{ct}", tag="s")
                nc.tensor.matmul(
                    g2ps, lhsT=mt8_sb[:, :, ct * P:(ct + 1) * P],
                    rhs=hf8[:, :, ks], start=True, stop=True, perf_mode=DR,
                )
                nc.vector.tensor_scalar_mul(
                    out=g2f8[:, ct, ks], in0=g2ps, scalar1=1.0 / 16.0,
                )

        def emit_vv_pair(i):
            kt0, kt1 = 2 * i, 2 * i + 1
            vps = {}
            for kt in (kt0, kt1):
                vps[kt] = ps_o.tile([P, 257], F32, name=f"vps_{kt}", tag="o")
                ks = slice(kt * P, (kt + 1) * P)
                nc.tensor.matmul(vps[kt], lhsT=hf8[:, :, ks], rhs=w2t8_sb,
                                 start=True, stop=True, perf_mode=DR)
            pr = slice(kt0, kt1 + 1)
            for kt in (kt0, kt1):
                nc.vector.tensor_scalar_mul(
                    out=fz[:, kt:kt + 1], in0=vps[kt][:, 256:257],
                    scalar1=float(SCALE / 16.0),
                )
            # f = exp(z), |z| < ~0.1: 1 + z + z^2/2 on DVE (keeps ACT free)
            nc.vector.tensor_scalar(
                out=ftmp[:, pr], in0=fz[:, pr], scalar1=0.5, scalar2=1.0,
                op0=ALU.mult, op1=ALU.add,
            )
            nc.vector.tensor_mul(out=fex[:, pr], in0=ftmp[:, pr], in1=fz[:, pr])
            nc.vector.tensor_scalar_add(out=fex[:, pr], in0=fex[:, pr], scalar1=1.0)
            for kt in (kt0, kt1):
                nc.vector.tensor_scalar_mul(
                    out=vv8[:, kt, 0:256], in0=vps[kt][:, 0:256],
                    scalar1=fex[:, kt:kt + 1],
                )
                nc.vector.tensor_scalar_mul(
                    out=vv8[:, kt, 256:257], in0=fex[:, kt:kt + 1], scalar1=1.0
                )

        # ---- attention ----
        e4_tiles = {}
        o_cur = {}
        tps_cur = {}
        outt_cur = {}

        def emit_s_pair(qb, t):
            qs_ = slice(qb * QB, (qb + 1) * QB)
            sp = ps_s.tile([P, 2, QB], F32, name=f"sps_{qb}_{t}", tag="s")
            for j in (0, 1):
                kt = 2 * t + j
                nc.tensor.matmul(
                    sp[:, j, :], lhsT=g2f8[:, :, kt * P:(kt + 1) * P],
                    rhs=hf8[:, :, qs_], start=True, stop=True, perf_mode=DR,
                )
            nc.scalar.activation(
                out=e4_tiles[qb][:, 2 * t:2 * t + 2, :], in_=sp,
                func=AF.Exp, scale=float(SCALE), bias=nshift,
            )

        def emit_pv(qb, qs, t):
            if t == 0:
                o_cur[qs] = ps_o.tile([P, 257], F32, name=f"ops_{qb}_{qs}", tag="o")
            nc.tensor.matmul(
                o_cur[qs],
                lhsT=e4_tiles[qb][:, 2 * t:2 * t + 2, qs * P:(qs + 1) * P],
                rhs=vv8[:, 2 * t:2 * t + 2, :],
                start=(t == 0), stop=(t == NPR - 1), perf_mode=DR,
            )

        def emit_qs_epilogue(qb, qs):
            o = o_cur[qs]
            recip = sml.tile([P, 1], F32, name=f"rc_{qb}_{qs}", tag="recip")
            nc.vector.reciprocal(out=recip, in_=o[:, 256:257])
            attn = anp.tile([P, C], BF16, name=f"attn_{qb}_{qs}", tag="attn")
            nc.vector.tensor_scalar_mul(out=attn, in0=o[:, 0:256], scalar1=recip)
            tps = ps_o.tile([P, CT, P], BF16, name=f"tps_{qb}_{qs}", tag="o")
            tps_cur[qs] = tps
            for ct in range(CT):
                nc.tensor.transpose(
                    tps[:, ct, :], attn[:, ct * P:(ct + 1) * P], eyeb,
                )
            outt = outt_cur[qb]
            n0 = qb * QB + qs * P
            for ct in range(CT):
                # attn carries a 16x scale from the fp8 VV encoding
                nc.vector.tensor_scalar(
                    out=outt[:, ct, qs * P:(qs + 1) * P], in0=tps[:, ct, :],
                    scalar1=1.0 / 16.0, scalar2=bo_sb[:, ct:ct + 1],
                    op0=ALU.mult, op1=ALU.add,
                )
                nc.vector.tensor_add(
                    out=outt[:, ct, qs * P:(qs + 1) * P],
                    in0=outt[:, ct, qs * P:(qs + 1) * P],
                    in1=x_sb[:, ct, n0:n0 + P],
                )

        def emit_qb_out(qb):
            outt = outt_cur.pop(qb)
            qs_ = slice(qb * QB, (qb + 1) * QB)
            out_r = out_d.rearrange("(t p) n -> p t n", p=P)
            nc.gpsimd.dma_start(out=out_r[:, :, qs_], in_=outt)

        def aux_pv(qb_prev, i):  # i in 0..15 -> 4 PV matmuls per step
            for k in range(4):
                idx = 4 * i + k
                qs, t = divmod(idx, NPR)
                if qs == 0 and t == 0:
                    outt_cur[qb_prev] = outp.tile(
                        [P, CT, QB], F32, name=f"outt_{qb_prev}", tag="outt"
                    )
                emit_pv(qb_prev, qs, t)
                if t == NPR - 1:
                    emit_qs_epilogue(qb_prev, qs)
                    if qs == 3:
                        emit_qb_out(qb_prev)

        for qb in range(NQB):
            e4_tiles[qb] = e4p.tile([P, NKT, QB], F8, name=f"e4_{qb}", tag="e4")
            if qb >= 2:
                del e4_tiles[qb - 2]
            for t in range(NPR):
                if qb == 0:
                    if t % 2 == 0:
                        emit_hg2(t // 2)
                    emit_s_pair(qb, t)
                    emit_vv_pair(t)
                else:
                    emit_s_pair(qb, t)
                    aux_pv(qb - 1, t)
        for i in range(NPR):
            aux_pv(NQB - 1, i)

    nc.compile()
    return nc


_NC = None


def _get_nc():
    global _NC
    if _NC is None:
        _NC = build_nc()
    return _NC


def _host_prep(x, w_q, b_q, w_k, b_k, w_v, b_v, w_o, b_o):
    x = np.ascontiguousarray(np.asarray(x, np.float32))
    B = x.shape[0]
    wq = np.asarray(w_q, np.float32)
    wk = np.asarray(w_k, np.float32)
    wv = np.asarray(w_v, np.float32)
    wo = np.asarray(w_o, np.float32)
    bq = np.asarray(b_q, np.float32)
    bv = np.asarray(b_v, np.float32)
    bo = np.asarray(b_o, np.float32)

    def to_pt(a):  # [C, ...] -> [P, CT, ...]
        return np.ascontiguousarray(
            a.reshape(CT, P, *a.shape[1:]).transpose(1, 0, *range(2, a.ndim + 1))
        )

    mt = (wk.T @ wq).astype(np.float32)       # lhsT[c, c'] = M[c', c]
    mt8 = to_pt((16.0 * mt).astype(F8NP))
    u = (wk.T @ bq).astype(np.float32)
    w2 = (wo @ wv).astype(np.float32)
    b2 = (wo @ bv).astype(np.float32)
    w2t = np.zeros((C, 257), np.float32)
    w2t[:, :256] = 16.0 * w2.T
    w2t[:, 256] = 16.0 * u
    w2t8 = to_pt(w2t.astype(F8NP))
    bo_eff = bo + b2   # sum_k softmax = 1 -> Wo b_v folds into the output bias

    xr = x.reshape(B, C, N)
    shared = {"mt8": mt8, "w2t8": w2t8, "bo": to_pt(bo_eff)}
    in_maps = [{"x": np.ascontiguousarray(xr[i]), **shared} for i in range(B)]
    return x, in_maps


def kernel(x, w_q, b_q, w_k, b_k, w_v, b_v, w_o, b_o):
    x, in_maps = _host_prep(x, w_q, b_q, w_k, b_k, w_v, b_v, w_o, b_o)
    B = x.shape[0]
    nc = _get_nc()
    res = run_bass_kernel_spmd(nc, in_maps, core_ids=list(range(B)))
    global _LAST
    _LAST = res
    out = np.stack([res.results[i]["out"] for i in range(B)], axis=0)
    return out.reshape(x.shape).astype(np.float32)


_LAST = None


# revision 9
# speedup vs baseline: 1.4688x; 1.2001x over previous
"""AttentionBlock (GroupNorm + single-head full attention + residual) on 8 TRN2 cores.

Data-parallel: batch B=8, one sample per NeuronCore. Per core:
  x [256, 4096] f32 -> groupnorm -> h (fp8 e4m3)
  Algebraic folding (host-precomputed weight products):
    S[q,k] = q.k = sum_c h[c,q]*G2[c,k] + w[k] + c0
       G2 = M h + v,  M = Wq^T Wk, v = Wq^T b_k,  w[k] = (Wk^T b_q).h_k
    out_pre[q,co] = sum_k P[k,q]*VV[co,k],  VV = (Wo Wv) h   (proj_out folded;
       the Wo b_v term rides on the output bias since sum_k softmax = 1, and
       the c0 score offset cancels in softmax)
  All heavy matmuls run in fp8 e4m3 with MatmulPerfMode.DoubleRow (contraction
  over 2 k-subtiles per instruction, 2x PE throughput).  The per-k score bias
  w[k] is folded multiplicatively into VV (f[k] = exp(SCALE*w[k]),
  sum_k e*f*vv == sum_k (e*f)*vv), which makes the softmax-exp bias a constant
  (-SHIFT) so each ACT exp instruction can span two PSUM banks (1024 wide).
  The softmax denominator rides as an f-column of VV.  P^T layout [k, q] comes
  straight out of the S^T matmul so the 4096x4096 attention matrix is never
  transposed; only the final [4096, 256] attention output is transposed back
  to [c, n] via TensorE.
"""

import numpy as np
import ml_dtypes

import concourse.bacc as bacc
import concourse.bass as bass
import concourse.tile as tile
from concourse import mybir
from concourse.bass_utils import run_bass_kernel_spmd

F32 = mybir.dt.float32
BF16 = mybir.dt.bfloat16
F8 = mybir.dt.float8e4
AF = mybir.ActivationFunctionType
DR = mybir.MatmulPerfMode.DoubleRow
ALU = mybir.AluOpType
F8NP = ml_dtypes.float8_e4m3fn

C = 256          # channels
N = 4096         # spatial (64*64)
P = 128          # partitions
CT = C // P      # channel tiles (2)
NG = 8           # groups
GS = C // NG     # group size (32)
EPS = 1e-5
QB = 512         # queries per block
NQB = N // QB    # 8
NKT = N // P     # 32 k-tiles
NPR = NKT // 2   # 16 k-tile pairs
SCALE = 1.0 / np.sqrt(C)  # 1/16
SHIFT = 3.0      # global exp shift (softmax-invariant), keeps fp8 e in range


def _group_masks():
    g0 = np.zeros((P, NG), np.float32)
    g1 = np.zeros((P, NG), np.float32)
    for p in range(P):
        g0[p, p // GS] = 1.0
        g1[p, 4 + p // GS] = 1.0
    return g0, g1


def build_nc():
    nc = bacc.Bacc("TRN2", target_bir_lowering=False)

    x_d = nc.dram_tensor("x", [C, N], F32, kind="ExternalInput")
    mt8_d = nc.dram_tensor("mt8", [P, CT, C], F8, kind="ExternalInput")
    vb_d = nc.dram_tensor("vb", [P, CT], F32, kind="ExternalInput")
    w2t8_d = nc.dram_tensor("w2t8", [P, CT, 257], F8, kind="ExternalInput")
    bo_d = nc.dram_tensor("bo", [P, CT], F32, kind="ExternalInput")
    out_d = nc.dram_tensor("out", [C, N], F32, kind="ExternalOutput")

    g0_np, g1_np = _group_masks()
    g0_d = nc.inline_tensor(g0_np, name="g0c")
    g1_d = nc.inline_tensor(g1_np, name="g1c")
    gt0_d = nc.inline_tensor(np.ascontiguousarray(g0_np.T), name="gt0c")
    gt1_d = nc.inline_tensor(np.ascontiguousarray(g1_np.T), name="gt1c")
    eye_d = nc.inline_tensor(np.eye(P, dtype=np.float32), name="eyec")

    import contextlib
    with tile.TileContext(nc) as tc, contextlib.ExitStack() as ctx:
        cst = ctx.enter_context(tc.tile_pool(name="cst", bufs=1))
        big = ctx.enter_context(tc.tile_pool(name="big", bufs=1))
        e4p = ctx.enter_context(tc.tile_pool(name="e4p", bufs=2))
        anp = ctx.enter_context(tc.tile_pool(name="anp", bufs=4))
        outp = ctx.enter_context(tc.tile_pool(name="outp", bufs=2))
        sml = ctx.enter_context(tc.tile_pool(name="sml", bufs=2))
        ps_s = ctx.enter_context(tc.tile_pool(name="ps_s", bufs=3, space="PSUM"))
        ps_o = ctx.enter_context(tc.tile_pool(name="ps_o", bufs=2, space="PSUM"))

        # ---- const loads ----
        mt8_sb = cst.tile([P, CT, C], F8, name="mt8_sb")
        nc.sync.dma_start(out=mt8_sb, in_=mt8_d[:, :, :])
        w2t8_sb = cst.tile([P, CT, 257], F8, name="w2t8_sb")
        nc.sync.dma_start(out=w2t8_sb, in_=w2t8_d[:, :, :])
        vb_sb = cst.tile([P, CT], F32, name="vb_sb")
        nc.sync.dma_start(out=vb_sb, in_=vb_d[:, :])
        bo_sb = cst.tile([P, CT], F32, name="bo_sb")
        nc.sync.dma_start(out=bo_sb, in_=bo_d[:, :])

        eye_sb = cst.tile([P, P], F32, name="eye_sb")
        nc.sync.dma_start(out=eye_sb, in_=eye_d[:, :])
        eyeb = cst.tile([P, P], BF16, name="eyeb")
        nc.vector.tensor_copy(out=eyeb, in_=eye_sb)

        g0_sb = cst.tile([P, NG], F32, name="g0_sb")
        nc.sync.dma_start(out=g0_sb, in_=g0_d[:, :])
        g1_sb = cst.tile([P, NG], F32, name="g1_sb")
        nc.sync.dma_start(out=g1_sb, in_=g1_d[:, :])
        gt0_sb = cst.tile([NG, P], F32, name="gt0_sb")
        nc.sync.dma_start(out=gt0_sb, in_=gt0_d[:, :])
        gt1_sb = cst.tile([NG, P], F32, name="gt1_sb")
        nc.sync.dma_start(out=gt1_sb, in_=gt1_d[:, :])

        eps_sb = cst.tile([NG, 1], F32, name="eps_sb")
        nc.vector.memset(eps_sb, EPS)
        nshift = cst.tile([P, 1], F32, name="nshift")
        nc.vector.memset(nshift, -SHIFT)
        zbias = cst.tile([P, 1], F32, name="zbias")
        nc.vector.memset(zbias, 0.0)

        # ---- x load (4 chunks to parallelize DMA queues) ----
        x_sb = big.tile([P, CT, N], F32, name="x_sb")
        x_r = x_d.rearrange("(t p) n -> p t n", p=P)
        NXC = 2  # chunks per ct
        XC = N // NXC
        for ct in range(CT):
            for xc in range(NXC):
                xs = slice(xc * XC, (xc + 1) * XC)
                nc.sync.dma_start(out=x_sb[:, ct, xs], in_=x_r[:, ct, xs])

        # ---- groupnorm stats ----
        NSG = N // 512
        stats = sml.tile([P, CT, NSG, 6], F32, name="stats")
        mv = sml.tile([P, CT, 2], F32, name="mv")
        for ct in range(CT):
            for sg in range(NSG):
                nc.vector.bn_stats(
                    out=stats[:, ct, sg, :], in_=x_sb[:, ct, sg * 512:(sg + 1) * 512]
                )
            nc.vector.bn_aggr(out=mv[:, ct, :], in_=stats[:, ct, :, :])
        st3 = sml.tile([P, CT, 3], F32, name="st3")
        for ct in range(CT):
            nc.vector.tensor_copy(out=st3[:, ct, 0:2], in_=mv[:, ct, :])
            nc.vector.tensor_mul(
                out=st3[:, ct, 2:3], in0=mv[:, ct, 0:1], in1=mv[:, ct, 0:1]
            )
        gps = ps_s.tile([NG, 3], F32, name="gps", tag="s")
        nc.tensor.matmul(gps, lhsT=g0_sb, rhs=st3[:, 0, :], start=True, stop=False)
        nc.tensor.matmul(gps, lhsT=g1_sb, rhs=st3[:, 1, :], start=False, stop=True)
        gsb = sml.tile([NG, 3], F32, name="gsb")
        nc.vector.tensor_copy(out=gsb, in_=gps)
        gmean = sml.tile([NG, 1], F32, name="gmean")
        nc.vector.tensor_scalar_mul(out=gmean, in0=gsb[:, 0:1], scalar1=1.0 / GS)
        gtmp = sml.tile([NG, 1], F32, name="gtmp")
        nc.vector.tensor_add(out=gtmp, in0=gsb[:, 1:2], in1=gsb[:, 2:3])
        nc.vector.tensor_scalar_mul(out=gtmp, in0=gtmp, scalar1=1.0 / GS)
        gmsq = sml.tile([NG, 1], F32, name="gmsq")
        nc.vector.tensor_mul(out=gmsq, in0=gmean, in1=gmean)
        gvar = sml.tile([NG, 1], F32, name="gvar")
        nc.vector.tensor_sub(out=gvar, in0=gtmp, in1=gmsq)
        gstd = sml.tile([NG, 1], F32, name="gstd")
        nc.scalar.activation(out=gstd, in_=gvar, func=AF.Sqrt, bias=eps_sb)
        grstd = sml.tile([NG, 1], F32, name="grstd")
        nc.vector.reciprocal(out=grstd, in_=gstd)
        gpar = sml.tile([NG, 2], F32, name="gpar")
        nc.vector.tensor_copy(out=gpar[:, 0:1], in_=gmean)
        nc.vector.tensor_copy(out=gpar[:, 1:2], in_=grstd)
        mr_sb = sml.tile([P, CT, 2], F32, name="mr_sb")
        for ct, gt in ((0, gt0_sb), (1, gt1_sb)):
            bps = ps_s.tile([P, 2], F32, name=f"bps{ct}", tag="s")
            nc.tensor.matmul(bps, lhsT=gt, rhs=gpar, start=True, stop=True)
            nc.vector.tensor_copy(out=mr_sb[:, ct, :], in_=bps)

        # ---- h fp8 + G2, pipelined per 512-col block ----
        hf8 = big.tile([P, CT, N], F8, name="hf8")
        g2f8 = big.tile([P, CT, N], F8, name="g2f8")
        for kb in range(NQB):
            ks = slice(kb * QB, (kb + 1) * QB)
            for ct in range(CT):
                nc.vector.tensor_scalar(
                    out=hf8[:, ct, ks],
                    in0=x_sb[:, ct, ks],
                    scalar1=mr_sb[:, ct, 0:1],
                    scalar2=mr_sb[:, ct, 1:2],
                    op0=ALU.subtract,
                    op1=ALU.mult,
                )
            for ct in range(CT):
                g2ps = ps_s.tile([P, QB], F32, name=f"g2ps_{kb}_{ct}", tag="s")
                nc.tensor.matmul(
                    g2ps, lhsT=mt8_sb[:, :, ct * P:(ct + 1) * P],
                    rhs=hf8[:, :, ks], start=True, stop=True, perf_mode=DR,
                )
                nc.vector.tensor_scalar(
                    out=g2f8[:, ct, ks], in0=g2ps,
                    scalar1=1.0 / 16.0, scalar2=vb_sb[:, ct:ct + 1],
                    op0=ALU.mult, op1=ALU.add,
                )

        # ---- VV projection + f = exp(SCALE*w) folding ----
        # vv8[k, 0:256] = 16*VV[k, co]*f[k]; vv8[k, 256] = f[k]
        vv8 = big.tile([P, NKT, 257], F8, name="vv8")
        fz = big.tile([P, NKT], F32, name="fz")
        fex = big.tile([P, NKT], F32, name="fex")

        def emit_vv_pair(i):
            kt0, kt1 = 2 * i, 2 * i + 1
            vps = {}
            for kt in (kt0, kt1):
                vps[kt] = ps_o.tile([P, 257], F32, name=f"vps_{kt}", tag="o")
                ks = slice(kt * P, (kt + 1) * P)
                nc.tensor.matmul(vps[kt], lhsT=hf8[:, :, ks], rhs=w2t8_sb,
                                 start=True, stop=True, perf_mode=DR)
            for kt in (kt0, kt1):
                nc.vector.tensor_scalar_mul(
                    out=fz[:, kt:kt + 1], in0=vps[kt][:, 256:257],
                    scalar1=float(SCALE / 16.0),
                )
            nc.scalar.activation(
                out=fex[:, kt0:kt1 + 1], in_=fz[:, kt0:kt1 + 1],
                func=AF.Exp, bias=zbias,
            )
            for kt in (kt0, kt1):
                nc.vector.tensor_scalar_mul(
                    out=vv8[:, kt, 0:256], in0=vps[kt][:, 0:256],
                    scalar1=fex[:, kt:kt + 1],
                )
                nc.vector.tensor_scalar_mul(
                    out=vv8[:, kt, 256:257], in0=fex[:, kt:kt + 1], scalar1=1.0
                )

        # ---- attention ----
        e4_tiles = {}
        o_cur = {}
        tps_cur = {}

        def emit_s_pair(qb, t):
            qs_ = slice(qb * QB, (qb + 1) * QB)
            sp = ps_s.tile([P, 2, QB], F32, name=f"sps_{qb}_{t}", tag="s")
            for j in (0, 1):
                kt = 2 * t + j
                nc.tensor.matmul(
                    sp[:, j, :], lhsT=g2f8[:, :, kt * P:(kt + 1) * P],
                    rhs=hf8[:, :, qs_], start=True, stop=True, perf_mode=DR,
                )
            nc.scalar.activation(
                out=e4_tiles[qb][:, 2 * t:2 * t + 2, :], in_=sp,
                func=AF.Exp, scale=float(SCALE), bias=nshift,
            )

        def emit_pv(qb, qs, t):
            if t == 0:
                o_cur[qs] = ps_o.tile([P, 257], F32, name=f"ops_{qb}_{qs}", tag="o")
            nc.tensor.matmul(
                o_cur[qs],
                lhsT=e4_tiles[qb][:, 2 * t:2 * t + 2, qs * P:(qs + 1) * P],
                rhs=vv8[:, 2 * t:2 * t + 2, :],
                start=(t == 0), stop=(t == NPR - 1), perf_mode=DR,
            )

        def emit_qs_epilogue(qb, qs):
            o = o_cur[qs]
            recip = sml.tile([P, 1], F32, name=f"rc_{qb}_{qs}", tag="recip")
            nc.vector.reciprocal(out=recip, in_=o[:, 256:257])
            recip2 = sml.tile([P, 1], F32, name=f"rc2_{qb}_{qs}", tag="recip2")
            nc.vector.tensor_scalar_mul(out=recip2, in0=recip, scalar1=1.0 / 16.0)
            attn = anp.tile([P, C], BF16, name=f"attn_{qb}_{qs}", tag="attn")
            nc.vector.tensor_scalar_mul(out=attn, in0=o[:, 0:256], scalar1=recip2)
            tps = ps_o.tile([P, CT, P], BF16, name=f"tps_{qb}_{qs}", tag="o")
            tps_cur[qs] = tps
            for ct in range(CT):
                nc.tensor.transpose(
                    tps[:, ct, :], attn[:, ct * P:(ct + 1) * P], eyeb,
                )
            # out tile slice for this qs
            outt = outt_cur[qb]
            n0 = qb * QB + qs * P
            for ct in range(CT):
                nc.vector.tensor_scalar_add(
                    out=outt[:, ct, qs * P:(qs + 1) * P], in0=tps[:, ct, :],
                    scalar1=bo_sb[:, ct:ct + 1],
                )
                nc.vector.tensor_add(
                    out=outt[:, ct, qs * P:(qs + 1) * P],
                    in0=outt[:, ct, qs * P:(qs + 1) * P],
                    in1=x_sb[:, ct, n0:n0 + P],
                )

        outt_cur = {}

        def emit_qb_out(qb):
            outt = outt_cur.pop(qb)
            qs_ = slice(qb * QB, (qb + 1) * QB)
            out_r = out_d.rearrange("(t p) n -> p t n", p=P)
            nc.gpsimd.dma_start(out=out_r[:, :, qs_], in_=outt)

        def aux_pv(qb_prev, i):  # i in 0..15 -> 4 PV matmuls per step
            for k in range(4):
                idx = 4 * i + k
                qs, t = divmod(idx, NPR)
                if qs == 0 and t == 0:
                    outt_cur[qb_prev] = outp.tile(
                        [P, CT, QB], F32, name=f"outt_{qb_prev}", tag="outt"
                    )
                emit_pv(qb_prev, qs, t)
                if t == NPR - 1:
                    emit_qs_epilogue(qb_prev, qs)
                    if qs == 3:
                        emit_qb_out(qb_prev)

        for qb in range(NQB):
            e4_tiles[qb] = e4p.tile([P, NKT, QB], F8, name=f"e4_{qb}", tag="e4")
            if qb >= 2:
                del e4_tiles[qb - 2]
            for t in range(NPR):
                emit_s_pair(qb, t)
                if qb == 0:
                    emit_vv_pair(t)
                else:
                    aux_pv(qb - 1, t)
        for i in range(NPR):
            aux_pv(NQB - 1, i)

    nc.compile()
    return nc


_NC = None


def _get_nc():
    global _NC
    if _NC is None:
        _NC = build_nc()
    return _NC


def _host_prep(x, w_q, b_q, w_k, b_k, w_v, b_v, w_o, b_o):
    x = np.ascontiguousarray(np.asarray(x, np.float32))
    B = x.shape[0]
    wq = np.asarray(w_q, np.float32)
    wk = np.asarray(w_k, np.float32)
    wv = np.asarray(w_v, np.float32)
    wo = np.asarray(w_o, np.float32)
    bq = np.asarray(b_q, np.float32)
    bk = np.asarray(b_k, np.float32)
    bv = np.asarray(b_v, np.float32)
    bo = np.asarray(b_o, np.float32)

    def to_pt(a):  # [C, ...] -> [P, CT, ...]
        return np.ascontiguousarray(
            a.reshape(CT, P, *a.shape[1:]).transpose(1, 0, *range(2, a.ndim + 1))
        )

    mt = (wk.T @ wq).astype(np.float32)       # lhsT[c, c'] = M[c', c]
    mt8 = to_pt((16.0 * mt).astype(F8NP))
    v = (wq.T @ bk).astype(np.float32)
    u = (wk.T @ bq).astype(np.float32)
    w2 = (wo @ wv).astype(np.float32)
    b2 = (wo @ bv).astype(np.float32)
    w2t = np.zeros((C, 257), np.float32)
    w2t[:, :256] = 16.0 * w2.T
    w2t[:, 256] = 16.0 * u
    w2t8 = to_pt(w2t.astype(F8NP))
    bo_eff = bo + b2   # sum_k softmax = 1 -> Wo b_v folds into the output bias

    xr = x.reshape(B, C, N)
    shared = {
        "mt8": mt8, "vb": to_pt(v), "w2t8": w2t8, "bo": to_pt(bo_eff),
    }
    in_maps = [{"x": np.ascontiguousarray(xr[i]), **shared} for i in range(B)]
    return x, in_maps


def kernel(x, w_q, b_q, w_k, b_k, w_v, b_v, w_o, b_o):
    x, in_maps = _host_prep(x, w_q, b_q, w_k, b_k, w_v, b_v, w_o, b_o)
    B = x.shape[0]
    nc = _get_nc()
    res = run_bass_kernel_spmd(nc, in_maps, core_ids=list(range(B)))
    global _LAST
    _LAST = res
    out = np.stack([res.results[i]["out"] for i in range(B)], axis=0)
    return out.reshape(x.shape).astype(np.float32)


_LAST = None
